# revision 16
# baseline (speedup 1.0000x reference)
"""MoE layer (top-2 of 8 experts, d_model=1024, d_hidden=512) on 8 trn2 cores.

Expert-parallel sparse dispatch: routing (gating logits, top-2, softmax) runs
on the host in fp32 as part of the dispatch step; only the tokens actually
routed to an expert are sent to that expert's core, so each core does ~1/4 of
the dense per-expert FLOPs the reference materializes.

Each core processes a fixed capacity of tokens split into a main segment
(its own expert, main_t tiles of 128) and one overflow tile that can carry
another expert's excess (keeps the SPMD program uniform while balancing
load: per-core work = max 17 tiles = 2176 tokens instead of padding every
core to the largest expert count). Gate weights are applied on-device as a
per-partition scalar multiply; the host scatter-adds the two expert
contributions per token (indices within one expert segment are unique, so
fancy-index += is exact).

Layout notes (same as the dense baseline):
  - x arrives host-gathered and transposed per-core as xT [D, cap] fp16 so
    both MLP matmuls contract over the partition dimension.
  - mm1 produces hT [C, tokens] (expert W1 stationary), mm2 flips back to
    token-major y [tokens, D] (hT chunks stationary) so the gate is a
    per-partition [128,1] scalar and the output DMAs out in native layout.
"""

import os
import sys

import numpy as np

for _p in ("/opt/trn_rl_repo", "/root/.axon_site/_ro/trn_rl_repo"):
    if _p not in sys.path and os.path.isdir(_p):
        sys.path.append(_p)

P = 128
D_MODEL = 1024
C_HID = 512
N_EXP = 8
N_CORES = 8

KC = D_MODEL // P  # 8 contraction chunks over D
CC = C_HID // P    # 4 contraction chunks over C
NT = 512           # moving-dim chunk (tokens) for mm1
DH = 512           # moving-dim chunk (d_model) for mm2

_CACHE = {}


def _block_sizes(main_cap):
    """Main-segment token blocks: two small lead blocks, then 512s."""
    assert main_cap >= NT and main_cap % P == 0
    sizes = [P, 3 * P]
    left = main_cap - 4 * P
    while left >= NT:
        sizes.append(NT)
        left -= NT
    if left:
        sizes.append(left)
    return sizes

# set by test harness to capture profiling info
TRACE = False
LAST_RESULT = None


def _install_ntff_hook_shim():
    """Register the axon NTFF profile hook if the image's antenv lacks it.

    bass_utils resolves the hook via `antenv.axon_hooks`; when that module is
    absent, tracing silently degrades. The hook implementation itself ships
    with the axon boot package, so wire it up through sys.modules.
    """
    try:
        from antenv.axon_hooks import get_axon_ntff_profile_hook  # noqa: F401
        return  # real module present
    except ImportError:
        pass
    try:
        import types

        if "/root/.axon_site" not in sys.path and os.path.isdir("/root/.axon_site"):
            sys.path.append("/root/.axon_site")
        from trn_agent_boot.trn_boot import _ntff_profile_via_ctypes

        so_path = "/opt/axon/libaxon_pjrt.so"
        if not os.path.exists(so_path):
            return
        hook = _ntff_profile_via_ctypes(so_path)
        mod = types.ModuleType("antenv.axon_hooks")
        mod.get_axon_ntff_profile_hook = lambda: hook
        mod.set_axon_ntff_profile_hook = lambda h: None
        import antenv

        antenv.axon_hooks = mod
        sys.modules["antenv.axon_hooks"] = mod
    except Exception:
        pass


def _split_excess_waits(nc, mybir, maxw=1):
    """This walrus build accepts at most one semaphore wait per instruction.

    Tile emits instructions (notably the kernel-tail drain) with several
    waits; split the extras into preceding single-wait NoOps on the same
    engine — program order makes the chain equivalent.
    """
    for f in nc.m.functions:
        for bb in f.blocks:
            out = []
            changed = False
            for ins in bb.instructions:
                si = ins.sync_info
                waits = list(si.on_wait) if (si is not None and si.on_wait) else []
                if len(waits) > maxw:
                    extra, keep = waits[:-maxw], waits[-maxw:]
                    for ci in range(0, len(extra), maxw):
                        out.append(mybir.InstNoOp(
                            name=f"{ins.name}_ws{ci}",
                            sync_info=mybir.SyncInfo(
                                on_wait=list(extra[ci:ci + maxw]), on_update=[]
                            ),
                            engine=ins.engine,
                            bass_nofuse=True,
                        ))
                    si.on_wait = keep
                    changed = True
                out.append(ins)
            if changed:
                bb.instructions = out
    return nc


def _build_nc(main_t, ov_t):
    import concourse.bass as bass
    import concourse.mybir as mybir
    import concourse.tile as tile
    from contextlib import ExitStack

    dt = mybir.dt
    f32 = dt.float32
    f16 = dt.float16
    OP = mybir.AluOpType
    ACT = mybir.ActivationFunctionType

    capt = main_t + ov_t
    cap = capt * P
    main_cap = main_t * P
    assert main_cap % NT == 0 and 0 < ov_t * P <= NT

    nc = bass.Bass("TRN2", debug=False)

    # all inputs arrive host-permuted into the exact SBUF layout, so every
    # DMA is 128 partition-contiguous descriptors (KB-scale) instead of
    # thousands of 256B strided ones:
    #   xT[p, blk_off + kc*n + t]   = x[tok = blk_s + t, d = kc*128 + p]
    #   w1[p, cm*1024 + kc*128 + c] = W1[kc*128 + p, cm*128 + c]
    #   w2[p, cc*1024 + d]          = W2[cc*128 + p, d]
    xT = nc.dram_tensor("xT", [P, KC * cap], f16, kind="ExternalInput")
    w1a = nc.dram_tensor("w1a", [P, KC * C_HID], f16, kind="ExternalInput")
    w2a = nc.dram_tensor("w2a", [P, CC * D_MODEL], f16, kind="ExternalInput")
    w1b = nc.dram_tensor("w1b", [P, KC * C_HID], f16, kind="ExternalInput")
    w2b = nc.dram_tensor("w2b", [P, CC * D_MODEL], f16, kind="ExternalInput")
    gt = nc.dram_tensor("gt", [P, capt], f32, kind="ExternalInput")
    out = nc.dram_tensor("out", [cap, D_MODEL], f16, kind="ExternalOutput")

    with tile.TileContext(nc) as tc:
        with ExitStack() as ctx:
            cpool = ctx.enter_context(tc.tile_pool(name="cpool", bufs=1))
            opool = ctx.enter_context(tc.tile_pool(name="opool", bufs=4))
            psum_h = ctx.enter_context(
                tc.tile_pool(name="psum_h", bufs=4, space="PSUM"))
            psum_y = ctx.enter_context(
                tc.tile_pool(name="psum_y", bufs=4, space="PSUM"))

            xt_sb = cpool.tile([P, KC * cap], f16, name="xt_sb")
            ht_sb = cpool.tile([P, CC, cap], f16, name="ht_sb")
            # per-cm w1 tiles: deps are tile-granular, so the first matmul
            # must only wait for cm0's DMA, not all of w1
            w1a_sb = [cpool.tile([P, D_MODEL], f16, name=f"w1a{cm}_sb")
                      for cm in range(CC)]
            w1b_sb = [cpool.tile([P, D_MODEL], f16, name=f"w1b{cm}_sb")
                      for cm in range(CC)]
            w2a_sb = cpool.tile([P, CC * D_MODEL], f16, name="w2a_sb")
            w2b_sb = cpool.tile([P, CC * D_MODEL], f16, name="w2b_sb")
            gt_sb = cpool.tile([P, capt], f32, name="gt_sb")

            def dma_cols(sb, t, lo, hi):
                nc.sync.dma_start(sb[:, lo:hi], t[:, lo:hi])

            # blocks: (token_start, n_tokens, weight_slot); xT/xt_sb are
            # block-major: block (s, n) occupies columns [8s, 8s + 8n), laid
            # out kc-major within the block. The first two main blocks are
            # small (128/384 tokens) so the PE's first matmul only waits on
            # ~0.5MB of DMA (w1a cm0 chunk + 128 tokens of x), not 2MB.
            blocks = []
            s = 0
            for n in _block_sizes(main_cap):
                blocks.append((s, n, 0))
                s += n
            blocks.append((main_cap, ov_t * P, 1))

            def dma_x(s, n):
                dma_cols(xt_sb, xT, KC * s, KC * (s + n))

            def dma_w1(sbs, t, cm):
                nc.sync.dma_start(
                    sbs[cm][:], t[:, cm * D_MODEL:(cm + 1) * D_MODEL])

            # DMA triggers retire serially on the sync engine (~0.6us each),
            # so they are ordered exactly in PE-consumption order: w1a cm0
            # and the small first x block gate the first matmul.
            dma_w1(w1a_sb, w1a, 0)
            dma_x(blocks[0][0], blocks[0][1])
            dma_w1(w1a_sb, w1a, 1)
            dma_x(blocks[1][0], blocks[1][1])
            dma_w1(w1a_sb, w1a, 2)
            dma_w1(w1a_sb, w1a, 3)
            rest_main = blocks[2:-1]
            if rest_main:
                dma_x(rest_main[0][0], rest_main[0][1])
            dma_cols(w2a_sb, w2a, 0, 2 * D_MODEL)
            nc.sync.dma_start(gt_sb[:], gt[:])
            dma_cols(w2a_sb, w2a, 2 * D_MODEL, 4 * D_MODEL)
            for (s, n, _slot) in rest_main[1:]:
                dma_x(s, n)
            for cm in range(CC):
                dma_w1(w1b_sb, w1b, cm)
            dma_x(blocks[-1][0], blocks[-1][1])
            dma_cols(w2b_sb, w2b, 0, 4 * D_MODEL)

            w1s = [w1a_sb, w1b_sb]
            w2s = [w2a_sb, w2b_sb]

            def mm1(s, n, w1_sb):
                for cm in range(CC):
                    ps = psum_h.tile([P, NT], f32, name="ps_h", tag="psh")
                    for kc in range(KC):
                        nc.tensor.matmul(
                            ps[:, 0:n],
                            lhsT=w1_sb[cm][:, kc * P:(kc + 1) * P],
                            rhs=xt_sb[:, KC * s + kc * n:KC * s + (kc + 1) * n],
                            start=(kc == 0),
                            stop=(kc == KC - 1),
                        )
                    nc.scalar.activation(
                        ht_sb[:, cm, s:s + n], ps[:, 0:n], ACT.Relu)

            MAXTTB = max(_block_sizes(main_cap)) // P

            def mm2_block(s, n, w2_sb):
                # one out tile + one out DMA per block (fewer sync-engine
                # trigger instructions than per-tile DMAs)
                ttb = n // P
                o_sb = opool.tile([P, MAXTTB, D_MODEL], f16, name="o_sb", tag="o")
                for j in range(ttb):
                    tt = s // P + j
                    for dh in range(D_MODEL // DH):
                        ps = psum_y.tile([P, DH], f32, name="ps_y", tag="psy")
                        for cc in range(CC):
                            nc.tensor.matmul(
                                ps[:],
                                lhsT=ht_sb[:, cc, tt * P:(tt + 1) * P],
                                rhs=w2_sb[:, cc * D_MODEL + dh * DH:
                                          cc * D_MODEL + (dh + 1) * DH],
                                start=(cc == 0),
                                stop=(cc == CC - 1),
                            )
                        nc.vector.tensor_single_scalar(
                            o_sb[:, j, dh * DH:(dh + 1) * DH], ps[:],
                            gt_sb[:, tt:tt + 1], op=OP.mult)
                nc.sync.dma_start(
                    out[s:s + n, :].rearrange("(tt p) d -> p tt d", p=P),
                    o_sb[:, 0:ttb, :])

            # software pipeline: mm1(b) then mm2 of block b-1, so relu /
            # gate-mult / out-DMA of one block hide behind the next block's
            # matmuls and output traffic spreads across the whole kernel.
            prev = None
            for (s, n, slot) in blocks:
                mm1(s, n, w1s[slot])
                if prev is not None:
                    ps_, pn_, pslot_ = prev
                    mm2_block(ps_, pn_, w2s[pslot_])
                prev = (s, n, slot)
            ps_, pn_, pslot_ = prev
            mm2_block(ps_, pn_, w2s[pslot_])

    import concourse.mybir as mybir
    _split_excess_waits(nc, mybir)
    return nc


def _get_nc(main_t, ov_t):
    key = (main_t, ov_t)
    if key not in _CACHE:
        _CACHE[key] = _build_nc(main_t, ov_t)
    return _CACHE[key]


def kernel(**inputs) -> np.ndarray:
    global LAST_RESULT
    x = np.ascontiguousarray(np.asarray(inputs["x"], dtype=np.float32))
    Wg = np.ascontiguousarray(np.asarray(inputs["Wg"], dtype=np.float32))
    W1 = np.ascontiguousarray(np.asarray(inputs["W1"], dtype=np.float32))
    W2 = np.ascontiguousarray(np.asarray(inputs["W2"], dtype=np.float32))

    B, S, D = x.shape
    T = B * S
    xf = x.reshape(T, D)

    # ---- routing on host (fp32, same math as the reference gating)
    logits = xf @ Wg                       # [T, E] fp32
    r = np.arange(T)
    e1 = np.argmax(logits, axis=1)
    l2 = logits.copy()
    l2[r, e1] = -np.inf
    e2 = np.argmax(l2, axis=1)
    s1 = logits[r, e1]
    s2 = logits[r, e2]
    z = np.exp(s2 - s1)                    # s1 >= s2, so z in (0, 1]
    p1 = (1.0 / (1.0 + z)).astype(np.float32)
    p2 = (z / (1.0 + z)).astype(np.float32)

    idxs, gates = [], []
    for e in range(N_EXP):
        m1 = e1 == e
        idx = np.nonzero(m1 | (e2 == e))[0]
        idxs.append(idx)
        gates.append(np.where(m1[idx], p1[idx], p2[idx]).astype(np.float32))

    # main segment sized so every expert's excess fits in the 8 overflow
    # tiles (one 128-token tile per core)
    main_t = 16
    while sum(-(-max(0, len(ix) - main_t * P) // P) for ix in idxs) > N_CORES:
        main_t += 1
    main_cap = main_t * P
    ov_t = 1
    cap = (main_t + ov_t) * P

    # overflow chunks (expert, offset_into_idx, n), assigned one per core
    chunks = []
    for e in range(N_EXP):
        o = main_cap
        while o < len(idxs[e]):
            chunks.append((e, o, min(P, len(idxs[e]) - o)))
            o += P
    donors = list(chunks) + [None] * (N_CORES - len(chunks))

    # pre-permute into the SBUF layouts the kernel expects (see _build_nc)
    w1h = [np.ascontiguousarray(
        W1[e].astype(np.float16).reshape(KC, P, CC, P)
        .transpose(1, 2, 0, 3).reshape(P, KC * C_HID)) for e in range(N_EXP)]
    w2h = [np.ascontiguousarray(
        W2[e].astype(np.float16).reshape(CC, P, D_MODEL)
        .transpose(1, 0, 2).reshape(P, CC * D_MODEL)) for e in range(N_EXP)]
    zw1 = np.zeros((P, KC * C_HID), np.float16)
    zw2 = np.zeros((P, CC * D_MODEL), np.float16)

    block_sizes = _block_sizes(main_cap) + [ov_t * P]

    in_maps = []
    for c in range(N_CORES):
        n_own = min(len(idxs[c]), main_cap)
        xp = np.zeros((cap, D_MODEL), np.float16)
        gp = np.zeros(cap, np.float32)
        xp[:n_own] = xf[idxs[c][:n_own]]
        gp[:n_own] = gates[c][:n_own]
        if donors[c] is not None:
            e, o, n = donors[c]
            xp[main_cap:main_cap + n] = xf[idxs[e][o:o + n]]
            gp[main_cap:main_cap + n] = gates[e][o:o + n]
        # block-major transposed layout: [128, sum_b(8 * n_b)]
        pieces, s = [], 0
        for n in block_sizes:
            pieces.append(
                xp[s:s + n].reshape(n, KC, P).transpose(2, 1, 0).reshape(P, KC * n))
            s += n
        in_maps.append({
            "xT": np.ascontiguousarray(np.concatenate(pieces, axis=1)),
            "gt": np.ascontiguousarray(gp.reshape(main_t + ov_t, P).T),
            "w1a": w1h[c],
            "w2a": w2h[c],
            "w1b": w1h[donors[c][0]] if donors[c] is not None else zw1,
            "w2b": w2h[donors[c][0]] if donors[c] is not None else zw2,
        })

    from concourse.bass_utils import run_bass_kernel_spmd

    _install_ntff_hook_shim()
    nc = _get_nc(main_t, ov_t)
    res = run_bass_kernel_spmd(
        nc, in_maps, core_ids=list(range(N_CORES)), trace=TRACE
    )
    LAST_RESULT = res

    # ---- combine: scatter-add the (gate-scaled) expert outputs per token.
    # Indices are unique within each segment, so fancy += is exact.
    outf = np.zeros((T, D_MODEL), np.float32)
    for c in range(N_CORES):
        y = np.asarray(res.results[c]["out"]).astype(np.float32)
        n_own = min(len(idxs[c]), main_cap)
        outf[idxs[c][:n_own]] += y[:n_own]
        if donors[c] is not None:
            e, o, n = donors[c]
            outf[idxs[e][o:o + n]] += y[main_cap:main_cap + n]
    return outf.reshape(B, S, D)


# revision 21
# speedup vs baseline: 1.0190x; 1.0190x over previous
"""MoE layer (top-2 of 8 experts, d_model=1024, d_hidden=512) on 8 trn2 cores.

Expert-parallel sparse dispatch: routing (gating logits, top-2, softmax) runs
on the host in fp32 as part of the dispatch step; only the tokens actually
routed to an expert are sent to that expert's core, so each core does ~1/4 of
the dense per-expert FLOPs the reference materializes.

Each core processes a fixed capacity of tokens split into a main segment
(its own expert, main_t tiles of 128) and one overflow tile that can carry
another expert's excess (keeps the SPMD program uniform while balancing
load: per-core work = max 17 tiles = 2176 tokens instead of padding every
core to the largest expert count). Gate weights are applied on-device as a
per-partition scalar multiply; the host scatter-adds the two expert
contributions per token (indices within one expert segment are unique, so
fancy-index += is exact).

Layout notes (same as the dense baseline):
  - x arrives host-gathered and transposed per-core as xT [D, cap] fp16 so
    both MLP matmuls contract over the partition dimension.
  - mm1 produces hT [C, tokens] (expert W1 stationary), mm2 flips back to
    token-major y [tokens, D] (hT chunks stationary) so the gate is a
    per-partition [128,1] scalar and the output DMAs out in native layout.
"""

import os
import sys

import numpy as np

for _p in ("/opt/trn_rl_repo", "/root/.axon_site/_ro/trn_rl_repo"):
    if _p not in sys.path and os.path.isdir(_p):
        sys.path.append(_p)

P = 128
D_MODEL = 1024
C_HID = 512
N_EXP = 8
N_CORES = 8

KC = D_MODEL // P  # 8 contraction chunks over D
CC = C_HID // P    # 4 contraction chunks over C
NT = 512           # moving-dim chunk (tokens) for mm1
DH = 512           # moving-dim chunk (d_model) for mm2

_CACHE = {}


def _block_sizes(main_cap):
    """Main-segment token blocks: two small lead blocks, then 512s."""
    assert main_cap >= NT and main_cap % P == 0
    sizes = [P, 3 * P]
    left = main_cap - 4 * P
    while left >= NT:
        sizes.append(NT)
        left -= NT
    if left:
        sizes.append(left)
    return sizes

# set by test harness to capture profiling info
TRACE = False
LAST_RESULT = None


def _install_ntff_hook_shim():
    """Register the axon NTFF profile hook if the image's antenv lacks it.

    bass_utils resolves the hook via `antenv.axon_hooks`; when that module is
    absent, tracing silently degrades. The hook implementation itself ships
    with the axon boot package, so wire it up through sys.modules.
    """
    try:
        from antenv.axon_hooks import get_axon_ntff_profile_hook  # noqa: F401
        return  # real module present
    except ImportError:
        pass
    try:
        import types

        if "/root/.axon_site" not in sys.path and os.path.isdir("/root/.axon_site"):
            sys.path.append("/root/.axon_site")
        from trn_agent_boot.trn_boot import _ntff_profile_via_ctypes

        so_path = "/opt/axon/libaxon_pjrt.so"
        if not os.path.exists(so_path):
            return
        hook = _ntff_profile_via_ctypes(so_path)
        mod = types.ModuleType("antenv.axon_hooks")
        mod.get_axon_ntff_profile_hook = lambda: hook
        mod.set_axon_ntff_profile_hook = lambda h: None
        import antenv

        antenv.axon_hooks = mod
        sys.modules["antenv.axon_hooks"] = mod
    except Exception:
        pass


def _split_excess_waits(nc, mybir, maxw=1):
    """This walrus build accepts at most one semaphore wait per instruction.

    Tile emits instructions (notably the kernel-tail drain) with several
    waits; split the extras into preceding single-wait NoOps on the same
    engine — program order makes the chain equivalent.
    """
    for f in nc.m.functions:
        for bb in f.blocks:
            out = []
            changed = False
            for ins in bb.instructions:
                si = ins.sync_info
                waits = list(si.on_wait) if (si is not None and si.on_wait) else []
                if len(waits) > maxw:
                    extra, keep = waits[:-maxw], waits[-maxw:]
                    for ci in range(0, len(extra), maxw):
                        out.append(mybir.InstNoOp(
                            name=f"{ins.name}_ws{ci}",
                            sync_info=mybir.SyncInfo(
                                on_wait=list(extra[ci:ci + maxw]), on_update=[]
                            ),
                            engine=ins.engine,
                            bass_nofuse=True,
                        ))
                    si.on_wait = keep
                    changed = True
                out.append(ins)
            if changed:
                bb.instructions = out
    return nc


def _build_nc(main_t, ov_t):
    import concourse.bass as bass
    import concourse.mybir as mybir
    import concourse.tile as tile
    from contextlib import ExitStack

    dt = mybir.dt
    f32 = dt.float32
    f16 = dt.float16
    OP = mybir.AluOpType
    ACT = mybir.ActivationFunctionType

    capt = main_t + ov_t
    cap = capt * P
    main_cap = main_t * P
    assert main_cap % NT == 0 and 0 < ov_t * P <= NT

    nc = bass.Bass("TRN2", debug=False)

    # all inputs arrive host-permuted into the exact SBUF layout, so every
    # DMA is 128 partition-contiguous descriptors (KB-scale) instead of
    # thousands of 256B strided ones:
    #   xT[p, blk_off + kc*n + t]   = x[tok = blk_s + t, d = kc*128 + p]
    #   w1[p, cm*1024 + kc*128 + c] = W1[kc*128 + p, cm*128 + c]
    #   w2[p, cc*1024 + d]          = W2[cc*128 + p, d]
    xT = nc.dram_tensor("xT", [P, KC * cap], f16, kind="ExternalInput")
    w1a = nc.dram_tensor("w1a", [P, KC * C_HID], f16, kind="ExternalInput")
    w2a = nc.dram_tensor("w2a", [P, CC * D_MODEL], f16, kind="ExternalInput")
    w1b = nc.dram_tensor("w1b", [P, KC * C_HID], f16, kind="ExternalInput")
    w2b = nc.dram_tensor("w2b", [P, CC * D_MODEL], f16, kind="ExternalInput")
    gt = nc.dram_tensor("gt", [P, capt], f32, kind="ExternalInput")
    out = nc.dram_tensor("out", [cap, D_MODEL], f16, kind="ExternalOutput")

    with tile.TileContext(nc) as tc:
        with ExitStack() as ctx:
            cpool = ctx.enter_context(tc.tile_pool(name="cpool", bufs=1))
            opool = ctx.enter_context(tc.tile_pool(name="opool", bufs=4))
            psum = ctx.enter_context(
                tc.tile_pool(name="psum", bufs=8, space="PSUM"))

            xt_sb = cpool.tile([P, KC * cap], f16, name="xt_sb")
            ht_sb = cpool.tile([P, CC, cap], f16, name="ht_sb")
            # per-cm w1 tiles: deps are tile-granular, so the first matmul
            # must only wait for cm0's DMA, not all of w1
            w1a_tiles = [cpool.tile([P, D_MODEL], f16, name=f"w1a{cm}_sb")
                         for cm in range(CC)]
            w1b_flat = cpool.tile([P, KC * C_HID], f16, name="w1b_sb")

            def w1a_ap(cm, kc):
                return w1a_tiles[cm][:, kc * P:(kc + 1) * P]

            def w1b_ap(cm, kc):
                return w1b_flat[:, cm * D_MODEL + kc * P:
                                cm * D_MODEL + (kc + 1) * P]
            w2a_sb = cpool.tile([P, CC * D_MODEL], f16, name="w2a_sb")
            w2b_sb = cpool.tile([P, CC * D_MODEL], f16, name="w2b_sb")
            gt_sb = cpool.tile([P, capt], f32, name="gt_sb")

            def dma_cols(sb, t, lo, hi):
                nc.sync.dma_start(sb[:, lo:hi], t[:, lo:hi])

            # blocks: (token_start, n_tokens, weight_slot); xT/xt_sb are
            # block-major: block (s, n) occupies columns [8s, 8s + 8n), laid
            # out kc-major within the block. The first two main blocks are
            # small (128/384 tokens) so the PE's first matmul only waits on
            # ~0.5MB of DMA (w1a cm0 chunk + 128 tokens of x), not 2MB.
            blocks = []
            s = 0
            for n in _block_sizes(main_cap):
                blocks.append((s, n, 0))
                s += n
            blocks.append((main_cap, ov_t * P, 1))

            def dma_x(s, n):
                dma_cols(xt_sb, xT, KC * s, KC * (s + n))

            def dma_w1a(cm):
                nc.sync.dma_start(
                    w1a_tiles[cm][:], w1a[:, cm * D_MODEL:(cm + 1) * D_MODEL])

            # DMA triggers retire serially on the sync engine (~0.6us each),
            # so they are ordered exactly in PE-consumption order: w1a cm0
            # and the small first x block gate the first matmul.
            dma_w1a(0)
            dma_x(blocks[0][0], blocks[0][1])
            dma_w1a(1)
            dma_x(blocks[1][0], blocks[1][1])
            dma_w1a(2)
            dma_w1a(3)
            rest_main = blocks[2:-1]
            if rest_main:
                dma_x(rest_main[0][0], rest_main[0][1])
            dma_cols(w2a_sb, w2a, 0, 2 * D_MODEL)
            nc.sync.dma_start(gt_sb[:], gt[:])
            dma_cols(w2a_sb, w2a, 2 * D_MODEL, 4 * D_MODEL)
            for (s, n, _slot) in rest_main[1:]:
                dma_x(s, n)
            nc.sync.dma_start(w1b_flat[:], w1b[:])
            dma_x(blocks[-1][0], blocks[-1][1])
            dma_cols(w2b_sb, w2b, 0, 4 * D_MODEL)

            w1s = [w1a_ap, w1b_ap]
            w2s = [w2a_sb, w2b_sb]

            def mm1(s, n, w1_ap):
                for cm in range(CC):
                    ps = psum.tile([P, NT], f32, name="ps_h", tag="ps")
                    for kc in range(KC):
                        nc.tensor.matmul(
                            ps[:, 0:n],
                            lhsT=w1_ap(cm, kc),
                            rhs=xt_sb[:, KC * s + kc * n:KC * s + (kc + 1) * n],
                            start=(kc == 0),
                            stop=(kc == KC - 1),
                        )
                    nc.scalar.activation(
                        ht_sb[:, cm, s:s + n], ps[:, 0:n], ACT.Relu)

            def mm2(tt, w2_sb):
                o_sb = opool.tile([P, D_MODEL], f16, name="o_sb", tag="o")
                for dh in range(D_MODEL // DH):
                    ps = psum.tile([P, DH], f32, name="ps_y", tag="ps")
                    for cc in range(CC):
                        nc.tensor.matmul(
                            ps[:],
                            lhsT=ht_sb[:, cc, tt * P:(tt + 1) * P],
                            rhs=w2_sb[:, cc * D_MODEL + dh * DH:
                                      cc * D_MODEL + (dh + 1) * DH],
                            start=(cc == 0),
                            stop=(cc == CC - 1),
                        )
                    nc.vector.tensor_single_scalar(
                        o_sb[:, dh * DH:(dh + 1) * DH], ps[:],
                        gt_sb[:, tt:tt + 1], op=OP.mult)
                nc.sync.dma_start(out[tt * P:(tt + 1) * P, :], o_sb[:])

            # software pipeline: mm1(b) then mm2 of block b-1, so relu /
            # gate-mult / out-DMA of one block hide behind the next block's
            # matmuls and output traffic spreads across the whole kernel.
            prev = None
            for (s, n, slot) in blocks:
                mm1(s, n, w1s[slot])
                if prev is not None:
                    ps_, pn_, pslot_ = prev
                    for tt in range(ps_ // P, (ps_ + pn_) // P):
                        mm2(tt, w2s[pslot_])
                prev = (s, n, slot)
            ps_, pn_, pslot_ = prev
            for tt in range(ps_ // P, (ps_ + pn_) // P):
                mm2(tt, w2s[pslot_])

    import concourse.mybir as mybir
    _split_excess_waits(nc, mybir)
    return nc


def _get_nc(main_t, ov_t):
    key = (main_t, ov_t)
    if key not in _CACHE:
        _CACHE[key] = _build_nc(main_t, ov_t)
    return _CACHE[key]


def kernel(**inputs) -> np.ndarray:
    global LAST_RESULT
    x = np.ascontiguousarray(np.asarray(inputs["x"], dtype=np.float32))
    Wg = np.ascontiguousarray(np.asarray(inputs["Wg"], dtype=np.float32))
    W1 = np.ascontiguousarray(np.asarray(inputs["W1"], dtype=np.float32))
    W2 = np.ascontiguousarray(np.asarray(inputs["W2"], dtype=np.float32))

    B, S, D = x.shape
    T = B * S
    xf = x.reshape(T, D)

    # ---- routing on host (fp32, same math as the reference gating)
    logits = xf @ Wg                       # [T, E] fp32
    r = np.arange(T)
    e1 = np.argmax(logits, axis=1)
    l2 = logits.copy()
    l2[r, e1] = -np.inf
    e2 = np.argmax(l2, axis=1)
    s1 = logits[r, e1]
    s2 = logits[r, e2]
    z = np.exp(s2 - s1)                    # s1 >= s2, so z in (0, 1]
    p1 = (1.0 / (1.0 + z)).astype(np.float32)
    p2 = (z / (1.0 + z)).astype(np.float32)

    idxs, gates = [], []
    for e in range(N_EXP):
        m1 = e1 == e
        idx = np.nonzero(m1 | (e2 == e))[0]
        idxs.append(idx)
        gates.append(np.where(m1[idx], p1[idx], p2[idx]).astype(np.float32))

    # main segment sized so every expert's excess fits in the 8 overflow
    # tiles (one 128-token tile per core)
    main_t = 16
    while sum(-(-max(0, len(ix) - main_t * P) // P) for ix in idxs) > N_CORES:
        main_t += 1
    main_cap = main_t * P
    ov_t = 1
    cap = (main_t + ov_t) * P

    # overflow chunks (expert, offset_into_idx, n), assigned one per core
    chunks = []
    for e in range(N_EXP):
        o = main_cap
        while o < len(idxs[e]):
            chunks.append((e, o, min(P, len(idxs[e]) - o)))
            o += P
    donors = list(chunks) + [None] * (N_CORES - len(chunks))

    # pre-permute into the SBUF layouts the kernel expects (see _build_nc)
    w1h = [np.ascontiguousarray(
        W1[e].astype(np.float16).reshape(KC, P, CC, P)
        .transpose(1, 2, 0, 3).reshape(P, KC * C_HID)) for e in range(N_EXP)]
    w2h = [np.ascontiguousarray(
        W2[e].astype(np.float16).reshape(CC, P, D_MODEL)
        .transpose(1, 0, 2).reshape(P, CC * D_MODEL)) for e in range(N_EXP)]
    zw1 = np.zeros((P, KC * C_HID), np.float16)
    zw2 = np.zeros((P, CC * D_MODEL), np.float16)

    block_sizes = _block_sizes(main_cap) + [ov_t * P]

    in_maps = []
    for c in range(N_CORES):
        n_own = min(len(idxs[c]), main_cap)
        xp = np.zeros((cap, D_MODEL), np.float16)
        gp = np.zeros(cap, np.float32)
        xp[:n_own] = xf[idxs[c][:n_own]]
        gp[:n_own] = gates[c][:n_own]
        if donors[c] is not None:
            e, o, n = donors[c]
            xp[main_cap:main_cap + n] = xf[idxs[e][o:o + n]]
            gp[main_cap:main_cap + n] = gates[e][o:o + n]
        # block-major transposed layout: [128, sum_b(8 * n_b)]
        pieces, s = [], 0
        for n in block_sizes:
            pieces.append(
                xp[s:s + n].reshape(n, KC, P).transpose(2, 1, 0).reshape(P, KC * n))
            s += n
        in_maps.append({
            "xT": np.ascontiguousarray(np.concatenate(pieces, axis=1)),
            "gt": np.ascontiguousarray(gp.reshape(main_t + ov_t, P).T),
            "w1a": w1h[c],
            "w2a": w2h[c],
            "w1b": w1h[donors[c][0]] if donors[c] is not None else zw1,
            "w2b": w2h[donors[c][0]] if donors[c] is not None else zw2,
        })

    from concourse.bass_utils import run_bass_kernel_spmd

    _install_ntff_hook_shim()
    nc = _get_nc(main_t, ov_t)
    res = run_bass_kernel_spmd(
        nc, in_maps, core_ids=list(range(N_CORES)), trace=TRACE
    )
    LAST_RESULT = res

    # ---- combine: scatter-add the (gate-scaled) expert outputs per token.
    # Indices are unique within each segment, so fancy += is exact.
    outf = np.zeros((T, D_MODEL), np.float32)
    for c in range(N_CORES):
        y = np.asarray(res.results[c]["out"]).astype(np.float32)
        n_own = min(len(idxs[c]), main_cap)
        outf[idxs[c][:n_own]] += y[:n_own]
        if donors[c] is not None:
            e, o, n = donors[c]
            outf[idxs[e][o:o + n]] += y[main_cap:main_cap + n]
    return outf.reshape(B, S, D)
